# revision 10
# baseline (speedup 1.0000x reference)
"""DiGCN 2-layer forward on 8 Trainium2 NeuronCores.

Strategy (graph/edge parallelism, per sharding hint):
  - Node space padded to NPAD = 784*128. Destination nodes are split into
    784 windows of 128; core k owns 98 consecutive windows (its dst range)
    and produces the output rows for exactly those nodes (no all-reduce).
  - Layer semantics: out = segment_sum(ew * (x @ W)[src], dst) + b.
    Per core: dense transform x = emb @ W1 (replicated, f32 table in DRAM),
    then per dst-window: bulk-gather x[src] rows with dma_gather (int16
    indices, table split in 4 chunks of 25088 rows), build one-hot
    dst-selection matrices on DVE, and contract them against the weighted
    messages on the TensorEngine with PSUM accumulation (the segment-sum).
  - Between layers each core holds h only for its dst range; the layer-2
    table x2 = relu(h+b1) @ W2 is AllGather'd across the 8 cores.
  - Matmul operands in bf16, all accumulation in f32 PSUM.
"""

import numpy as np
import ml_dtypes

import concourse.bass as bass
import concourse.bacc as bacc
import concourse.mybir as mybir
import concourse.tile as tile
from concourse.bass_utils import run_bass_kernel_spmd
from concourse.masks import make_identity

P = 128
NCORES = 8
GRP = 2  # dst windows per gather group

F32 = mybir.dt.float32
BF16 = mybir.dt.bfloat16
I16 = mybir.dt.int16
BF16_NP = ml_dtypes.bfloat16


_NC_CACHE = {}


def build_nc(npad, wpc, chunk_rows, B, n_in, h1, h2):
    key = (npad, wpc, chunk_rows, tuple(B), n_in, h1, h2)
    if key in _NC_CACHE:
        return _NC_CACHE[key]
    nc = _build_nc(npad, wpc, chunk_rows, B, n_in, h1, h2)
    _NC_CACHE[key] = nc
    return nc


def _build_nc(npad, wpc, chunk_rows, B, n_in, h1, h2):
    """Build the SPMD bass program (identical on all cores; data differs).

    npad:       padded node count (multiple of 128*NCORES and 512)
    wpc:        dst windows per core
    chunk_rows: rows per gather chunk (<= 32768)
    B:          list of blocks-per-window for each chunk (B[j] >= 1)
    """
    nwin = npad // P
    assert nwin == wpc * NCORES and wpc % GRP == 0
    ngroups = npad // 512
    nodes_pc = wpc * P
    nch = len(B)
    assert nch * chunk_rows == npad
    C = sum(B)
    off = np.concatenate([[0], np.cumsum(B)])  # block offsets per chunk
    npairs = wpc // GRP
    W = GRP * C  # total blocks per pair-group

    nc = bacc.Bacc("TRN2", target_bir_lowering=False, debug=False,
                   enable_asserts=False, num_devices=NCORES)

    embT = nc.dram_tensor("embT", [n_in, npad], BF16, kind="ExternalInput")
    w1 = nc.dram_tensor("w1", [n_in, h1], F32, kind="ExternalInput")
    w2 = nc.dram_tensor("w2", [h1, h2], F32, kind="ExternalInput")
    b1 = nc.dram_tensor("b1", [1, h1], F32, kind="ExternalInput")
    b2 = nc.dram_tensor("b2", [1, h2], F32, kind="ExternalInput")
    idx = nc.dram_tensor("idx", [npairs, P, 16 * C], I16, kind="ExternalInput")
    ewdst = nc.dram_tensor("ewdst", [npairs, P, 2 * W], BF16, kind="ExternalInput")
    out2 = nc.dram_tensor("out2", [nodes_pc, h2], F32, kind="ExternalOutput")

    x1_table = nc.dram_tensor("x1_table", [npad, h1], F32)
    x2_slice = nc.dram_tensor("x2_slice", [nodes_pc, h2], F32)
    x2_table = nc.dram_tensor("x2_table", [npad, h2], F32, addr_space="Shared")

    with tile.TileContext(nc) as tc:
        with (
            tc.tile_pool(name="consts", bufs=1) as consts,
            tc.tile_pool(name="cpsum", bufs=1, space="PSUM") as cpsum,
        ):
            # ---- constants ----
            w1_f = consts.tile([n_in, h1], F32)
            nc.sync.dma_start(out=w1_f[:], in_=w1[:])
            w1_sb = consts.tile([n_in, h1], BF16)
            nc.vector.tensor_copy(out=w1_sb[:], in_=w1_f[:])

            w2_f = consts.tile([h1, h2], F32)
            nc.sync.dma_start(out=w2_f[:], in_=w2[:])
            w2_sb = consts.tile([h1, h2], BF16)
            nc.vector.tensor_copy(out=w2_sb[:], in_=w2_f[:])

            ident = consts.tile([P, P], BF16)
            make_identity(nc, ident[:])

            iota128 = consts.tile([P, P], BF16)
            nc.gpsimd.iota(iota128[:], pattern=[[1, P]], base=0,
                           channel_multiplier=0,
                           allow_small_or_imprecise_dtypes=True)

            # bias rows replicated to 128 partitions via K=1 matmul with ones
            ones_t = consts.tile([1, P], BF16)
            nc.vector.memset(ones_t[:], 1.0)
            b1_f = consts.tile([1, h1], F32)
            nc.sync.dma_start(out=b1_f[:], in_=b1[:])
            b1_bf = consts.tile([1, h1], BF16)
            nc.vector.tensor_copy(out=b1_bf[:], in_=b1_f[:])
            b2_f = consts.tile([1, h2], F32)
            nc.sync.dma_start(out=b2_f[:], in_=b2[:])
            b2_bf = consts.tile([1, h2], BF16)
            nc.vector.tensor_copy(out=b2_bf[:], in_=b2_f[:])

            brep_ps = cpsum.tile([P, h1 + h2], F32)
            nc.tensor.matmul(brep_ps[:, :h1], lhsT=ones_t[:], rhs=b1_bf[:],
                             start=True, stop=True)
            nc.tensor.matmul(brep_ps[:, h1:], lhsT=ones_t[:], rhs=b2_bf[:],
                             start=True, stop=True)
            b1rep = consts.tile([P, h1], BF16)
            nc.vector.tensor_copy(out=b1rep[:], in_=brep_ps[:, :h1])
            b2rep = consts.tile([P, h2], BF16)
            nc.vector.tensor_copy(out=b2rep[:], in_=brep_ps[:, h1:])

            # ---- phase 1: x1 = emb @ W1, full table, replicated per core ----
            with (
                tc.tile_pool(name="tf", bufs=3) as tf,
                tc.tile_pool(name="tfp", bufs=2, space="PSUM") as tfp,
            ):
                for g in range(ngroups):
                    embT_t = tf.tile([n_in, 512], BF16, tag="embT_t")
                    nc.sync.dma_start(out=embT_t[:],
                                      in_=embT[:, g * 512:(g + 1) * 512])
                    ps = tfp.tile([P, 4 * h1], F32, tag="tf_ps")
                    for i in range(4):
                        nc.tensor.matmul(ps[:, i * h1:(i + 1) * h1],
                                         lhsT=embT_t[:, i * P:(i + 1) * P],
                                         rhs=w1_sb[:], start=True, stop=True)
                    x1_sb = tf.tile([P, 4 * h1], F32, tag="x1_sb")
                    nc.vector.tensor_copy(out=x1_sb[:], in_=ps[:])
                    # rows 512g+128i+p  <->  partition p, run i
                    nc.sync.dma_start(
                        out=x1_table[512 * g:512 * (g + 1), :].rearrange(
                            "(i p) d -> p i d", p=P),
                        in_=x1_sb[:].rearrange("p (i d) -> p i d", i=4))

            # ---- shared window-group machinery ----
            def group_aggregate(gidx, table, pools):
                """dma_gather + message scaling + one-hot build for pair gidx.
                Returns (Gs, S) tiles covering GRP windows x C blocks."""
                wp, h = pools
                idx_t = wp.tile([P, 16 * C], I16, tag="idx")
                nc.sync.dma_start(out=idx_t[:], in_=idx[gidx])
                ewd_t = wp.tile([P, 2 * W], BF16, tag="ewd")
                nc.sync.dma_start(out=ewd_t[:], in_=ewdst[gidx])

                g_t = wp.tile([P, W * h], F32, tag="gath")
                for j in range(nch):
                    nbt = GRP * B[j]
                    base = GRP * off[j]
                    for k in range(0, nbt, 8):  # <=1024 idxs per dma_gather
                        nb = min(8, nbt - k)
                        nc.gpsimd.dma_gather(
                            out_ap=g_t[:, (base + k) * h:(base + k + nb) * h]
                            .rearrange("p (b d) -> p b d", d=h),
                            in_ap=table[j * chunk_rows:(j + 1) * chunk_rows, :],
                            idxs_ap=idx_t[:, 8 * (base + k):8 * (base + k + nb)],
                            num_idxs=nb * P,
                            num_idxs_reg=nb * P,
                            elem_size=h,
                        )

                gs_t = wp.tile([P, W * h], BF16, tag="gs")
                nc.vector.tensor_tensor(
                    out=gs_t[:].rearrange("p (c d) -> p c d", d=h),
                    in0=g_t[:].rearrange("p (c d) -> p c d", d=h),
                    in1=ewd_t[:, 0:W, None].to_broadcast([P, W, h]),
                    op=mybir.AluOpType.mult)
                s_t = wp.tile([P, W * P], BF16, tag="sel")
                nc.vector.tensor_tensor(
                    out=s_t[:].rearrange("p (c j) -> p c j", j=P),
                    in0=iota128[:, None, :].to_broadcast([P, W, P]),
                    in1=ewd_t[:, W:2 * W, None].to_broadcast([P, W, P]),
                    op=mybir.AluOpType.is_equal)
                return gs_t, s_t

            def window_qlist(s):
                return [GRP * off[j] + s * B[j] + c
                        for j in range(nch) for c in range(B[j])]

            def accumulate(pp, gs_t, s_t, s, bias_rep, h):
                ps_z = pp.tile([P, h], F32, tag="ps_z")
                nc.tensor.matmul(ps_z[:], lhsT=ident[:], rhs=bias_rep[:],
                                 start=True, stop=False)
                qs = window_qlist(s)
                for n, q in enumerate(qs):
                    nc.tensor.matmul(ps_z[:],
                                     lhsT=s_t[:, q * P:(q + 1) * P],
                                     rhs=gs_t[:, q * h:(q + 1) * h],
                                     start=False, stop=(n == len(qs) - 1))
                return ps_z

            # ---- phase 2: layer-1 aggregation + per-window x2 transform ----
            with (
                tc.tile_pool(name="l1", bufs=2) as l1,
                tc.tile_pool(name="l1s", bufs=2) as l1s,
                tc.tile_pool(name="l1p", bufs=2, space="PSUM") as l1p,
                tc.tile_pool(name="l1tp", bufs=2, space="PSUM") as l1tp,
            ):
                for gi in range(npairs):
                    gs_t, s_t = group_aggregate(gi, x1_table, (l1, h1))
                    for s in range(GRP):
                        wi = gi * GRP + s
                        ps_z = accumulate(l1p, gs_t, s_t, s, b1rep, h1)
                        h_sb = l1s.tile([P, h1], BF16, tag="h_sb")
                        nc.scalar.activation(
                            out=h_sb[:], in_=ps_z[:],
                            func=mybir.ActivationFunctionType.Relu)
                        hT_ps = l1tp.tile([h1, P], BF16, tag="hT_ps")
                        nc.tensor.transpose(out=hT_ps[:], in_=h_sb[:],
                                            identity=ident[:])
                        hT_sb = l1s.tile([h1, P], BF16, tag="hT_sb")
                        nc.vector.tensor_copy(out=hT_sb[:], in_=hT_ps[:])
                        x2_ps = l1tp.tile([P, h2], F32, tag="x2_ps")
                        nc.tensor.matmul(x2_ps[:], lhsT=hT_sb[:], rhs=w2_sb[:],
                                         start=True, stop=True)
                        x2_sb = l1s.tile([P, h2], F32, tag="x2_sb")
                        nc.vector.tensor_copy(out=x2_sb[:], in_=x2_ps[:])
                        nc.sync.dma_start(
                            out=x2_slice[wi * P:(wi + 1) * P, :], in_=x2_sb[:])

            # ---- AllGather x2 slices -> full x2 table on every core ----
            nc.gpsimd.collective_compute(
                "AllGather", mybir.AluOpType.bypass,
                replica_groups=[list(range(NCORES))],
                ins=[x2_slice[:]], outs=[x2_table[:]])

            # ---- phase 3: layer-2 aggregation -> final output ----
            with (
                tc.tile_pool(name="l2", bufs=2) as l2,
                tc.tile_pool(name="l2s", bufs=2) as l2s,
                tc.tile_pool(name="l2p", bufs=2, space="PSUM") as l2p,
            ):
                for gi in range(npairs):
                    gs_t, s_t = group_aggregate(gi, x2_table, (l2, h2))
                    for s in range(GRP):
                        wi = gi * GRP + s
                        ps_z = accumulate(l2p, gs_t, s_t, s, b2rep, h2)
                        o_sb = l2s.tile([P, h2], F32, tag="o_sb")
                        nc.vector.tensor_copy(out=o_sb[:], in_=ps_z[:])
                        nc.sync.dma_start(out=out2[wi * P:(wi + 1) * P, :],
                                          in_=o_sb[:])

    nc.compile()
    return nc


def prep_inputs(edge_index, edge_weight, emb, W1, b1, W2, b2, npad, nch):
    """Host-side sharding: edges keyed by (dst window, src chunk), laid out
    into the per-pair-group block/partition slots the program expects."""
    wpc = npad // (P * NCORES)
    nwin = npad // P
    chunk_rows = npad // nch
    assert chunk_rows <= 32768
    n_in = emb.shape[1]
    src = np.asarray(edge_index[0], dtype=np.int64)
    dst = np.asarray(edge_index[1], dtype=np.int64)
    ew = np.asarray(edge_weight, dtype=np.float32)
    e = src.shape[0]

    w = dst >> 7
    j = src // chunk_rows
    key = w * nch + j
    cnt = np.bincount(key, minlength=nwin * nch).reshape(nwin, nch)
    B = [max(1, int(-(-cnt[:, jj].max() // P))) for jj in range(nch)]
    C = sum(B)
    off = np.concatenate([[0], np.cumsum(B)]).astype(np.int64)
    npairs_total = nwin // GRP
    Wt = GRP * C

    order = np.argsort(key, kind="stable")
    sk = key[order]
    starts = np.zeros(nwin * nch + 1, dtype=np.int64)
    starts[1:] = np.cumsum(cnt.reshape(-1))
    rank = np.arange(e, dtype=np.int64) - starts[sk]

    sw, sj = w[order], j[order]
    pg = sw // GRP           # pair-group id
    s = sw % GRP             # window within pair
    cc = rank // P           # block within (window, chunk)
    pp = rank % P
    q = GRP * off[sj] + s * B_arr(B)[sj] + cc

    ew_arr = np.zeros((npairs_total, P, 2 * Wt), np.float32)
    ew_arr[pg, pp, q] = ew[order]
    ew_arr[pg, pp, Wt + q] = (dst[order] % P).astype(np.float32)
    ewdst = ew_arr.astype(BF16_NP)

    # idx array: per (pair, chunk): flat order i = (s*B_j + c)*128 + p,
    # wrapped to [i % 16, 16*off_j + i // 16], replicated to 128 partitions.
    i_flat = (s * B_arr(B)[sj] + cc) * P + pp
    idx16 = np.zeros((npairs_total, 16, 8 * GRP * C), np.int16)
    idx16[pg, i_flat % 16, 8 * GRP * off[sj] + i_flat // 16] = \
        (src[order] - sj * chunk_rows).astype(np.int16)
    idx_full = np.tile(idx16, (1, 8, 1))

    embp = np.zeros((npad, n_in), np.float32)
    embp[:emb.shape[0]] = emb
    embT_bf = np.ascontiguousarray(embp.T).astype(BF16_NP)

    common = {
        "embT": embT_bf,
        "w1": np.asarray(W1, np.float32),
        "w2": np.asarray(W2, np.float32),
        "b1": np.asarray(b1, np.float32).reshape(1, -1),
        "b2": np.asarray(b2, np.float32).reshape(1, -1),
    }
    ppc = wpc // GRP  # pairs per core
    in_maps = []
    for k in range(NCORES):
        lo, hi = k * ppc, (k + 1) * ppc
        in_maps.append(dict(common,
                            idx=np.ascontiguousarray(idx_full[lo:hi]),
                            ewdst=np.ascontiguousarray(ewdst[lo:hi])))
    return in_maps, chunk_rows, B


def B_arr(B):
    return np.asarray(B, dtype=np.int64)


def run(edge_index, edge_weight, emb, W1, b1, W2, b2, npad, nch):
    n = emb.shape[0]
    wpc = npad // (P * NCORES)
    in_maps, chunk_rows, B = prep_inputs(
        edge_index, edge_weight, emb, W1, b1, W2, b2, npad, nch)
    nc = build_nc(npad, wpc, chunk_rows, B, emb.shape[1], W1.shape[1],
                  W2.shape[1])
    res = run_bass_kernel_spmd(nc, in_maps, core_ids=list(range(NCORES)))
    out = np.concatenate([res.results[k]["out2"] for k in range(NCORES)], axis=0)
    return out[:n]


def kernel(edge_index, edge_weight, emb, W1, b1, W2, b2):
    # 784 windows of 128 nodes; gather chunks of 25088 rows (int16-indexable)
    return run(edge_index, edge_weight, emb, W1, b1, W2, b2,
               npad=100352, nch=4)


# revision 11
# speedup vs baseline: 3.8741x; 3.8741x over previous
"""DiGCN 2-layer forward on 8 Trainium2 NeuronCores.

Strategy (graph/edge parallelism, per sharding hint):
  - Node space padded to NPAD = 784*128. Destination nodes are split into
    784 windows of 128; core k owns 98 consecutive windows (its dst range)
    and produces the output rows for exactly those nodes (no all-reduce).
  - Layer semantics: out = segment_sum(ew * (x @ W)[src], dst) + b.
    Per core: dense transform x = emb @ W1 (replicated, f32 table in DRAM),
    then per dst-window: bulk-gather x[src] rows with dma_gather (int16
    indices, table split in 4 chunks of 25088 rows), build one-hot
    dst-selection matrices on DVE, and contract them against the weighted
    messages on the TensorEngine with PSUM accumulation (the segment-sum).
  - Between layers each core holds h only for its dst range; the layer-2
    table x2 = relu(h+b1) @ W2 is AllGather'd across the 8 cores.
  - Matmul operands in bf16, all accumulation in f32 PSUM.
"""

import numpy as np
import ml_dtypes

import concourse.bass as bass
import concourse.bacc as bacc
import concourse.mybir as mybir
import concourse.tile as tile
from concourse.bass_utils import run_bass_kernel_spmd
from concourse.masks import make_identity

P = 128
NCORES = 8
GRP = 2  # dst windows per gather group

F32 = mybir.dt.float32
BF16 = mybir.dt.bfloat16
I16 = mybir.dt.int16
BF16_NP = ml_dtypes.bfloat16


_NC_CACHE = {}


def build_nc(npad, wpc, chunk_rows, B, n_in, h1, h2):
    key = (npad, wpc, chunk_rows, tuple(B), n_in, h1, h2)
    if key in _NC_CACHE:
        return _NC_CACHE[key]
    nc = _build_nc(npad, wpc, chunk_rows, B, n_in, h1, h2)
    _NC_CACHE[key] = nc
    return nc


def _build_nc(npad, wpc, chunk_rows, B, n_in, h1, h2):
    """Build the SPMD bass program (identical on all cores; data differs).

    npad:       padded node count (multiple of 128*NCORES and 512)
    wpc:        dst windows per core
    chunk_rows: rows per gather chunk (<= 32768)
    B:          list of blocks-per-window for each chunk (B[j] >= 1)
    """
    nwin = npad // P
    assert nwin == wpc * NCORES and wpc % GRP == 0
    ngroups = npad // 512
    nodes_pc = wpc * P
    nch = len(B)
    assert nch * chunk_rows == npad
    C = sum(B)
    off = np.concatenate([[0], np.cumsum(B)])  # block offsets per chunk
    npairs = wpc // GRP
    W = GRP * C  # total blocks per pair-group

    nc = bacc.Bacc("TRN2", target_bir_lowering=False, debug=False,
                   enable_asserts=False, num_devices=NCORES)

    embT = nc.dram_tensor("embT", [n_in, npad], BF16, kind="ExternalInput")
    w1 = nc.dram_tensor("w1", [n_in, h1], F32, kind="ExternalInput")
    w2 = nc.dram_tensor("w2", [h1, h2], F32, kind="ExternalInput")
    b1 = nc.dram_tensor("b1", [1, h1], F32, kind="ExternalInput")
    b2 = nc.dram_tensor("b2", [1, h2], F32, kind="ExternalInput")
    idx = nc.dram_tensor("idx", [npairs, P, 16 * C], I16, kind="ExternalInput")
    ewdst = nc.dram_tensor("ewdst", [npairs, P, 2 * W], BF16, kind="ExternalInput")
    out2 = nc.dram_tensor("out2", [nodes_pc, h2], F32, kind="ExternalOutput")

    x1_table = nc.dram_tensor("x1_table", [npad, h1], F32)
    x2_slice = nc.dram_tensor("x2_slice", [nodes_pc, h2], F32)
    x2_table = nc.dram_tensor("x2_table", [npad, h2], F32, addr_space="Shared")

    with tile.TileContext(nc) as tc:
        with (
            tc.tile_pool(name="consts", bufs=1) as consts,
            tc.tile_pool(name="cpsum", bufs=1, space="PSUM") as cpsum,
        ):
            # ---- constants ----
            w1_f = consts.tile([n_in, h1], F32)
            nc.sync.dma_start(out=w1_f[:], in_=w1[:])
            w1_sb = consts.tile([n_in, h1], BF16)
            nc.vector.tensor_copy(out=w1_sb[:], in_=w1_f[:])

            w2_f = consts.tile([h1, h2], F32)
            nc.sync.dma_start(out=w2_f[:], in_=w2[:])
            w2_sb = consts.tile([h1, h2], BF16)
            nc.vector.tensor_copy(out=w2_sb[:], in_=w2_f[:])

            ident = consts.tile([P, P], BF16)
            make_identity(nc, ident[:])

            iota128 = consts.tile([P, P], BF16)
            nc.gpsimd.iota(iota128[:], pattern=[[1, P]], base=0,
                           channel_multiplier=0,
                           allow_small_or_imprecise_dtypes=True)

            # bias rows replicated to 128 partitions via K=1 matmul with ones
            ones_t = consts.tile([1, P], BF16)
            nc.vector.memset(ones_t[:], 1.0)
            b1_f = consts.tile([1, h1], F32)
            nc.sync.dma_start(out=b1_f[:], in_=b1[:])
            b1_bf = consts.tile([1, h1], BF16)
            nc.vector.tensor_copy(out=b1_bf[:], in_=b1_f[:])
            b2_f = consts.tile([1, h2], F32)
            nc.sync.dma_start(out=b2_f[:], in_=b2[:])
            b2_bf = consts.tile([1, h2], BF16)
            nc.vector.tensor_copy(out=b2_bf[:], in_=b2_f[:])

            brep_ps = cpsum.tile([P, h1 + h2], F32)
            nc.tensor.matmul(brep_ps[:, :h1], lhsT=ones_t[:], rhs=b1_bf[:],
                             start=True, stop=True)
            nc.tensor.matmul(brep_ps[:, h1:], lhsT=ones_t[:], rhs=b2_bf[:],
                             start=True, stop=True)
            b1rep = consts.tile([P, h1], BF16)
            nc.vector.tensor_copy(out=b1rep[:], in_=brep_ps[:, :h1])
            b2rep = consts.tile([P, h2], BF16)
            nc.vector.tensor_copy(out=b2rep[:], in_=brep_ps[:, h1:])

            # ---- phase 1: x1 = emb @ W1, full table, replicated per core ----
            with (
                tc.tile_pool(name="tf", bufs=3) as tf,
                tc.tile_pool(name="tfp", bufs=2, space="PSUM") as tfp,
            ):
                for g in range(ngroups):
                    embT_t = tf.tile([n_in, 512], BF16, tag="embT_t")
                    nc.sync.dma_start(out=embT_t[:],
                                      in_=embT[:, g * 512:(g + 1) * 512])
                    ps = tfp.tile([P, 4 * h1], F32, tag="tf_ps")
                    for i in range(4):
                        nc.tensor.matmul(ps[:, i * h1:(i + 1) * h1],
                                         lhsT=embT_t[:, i * P:(i + 1) * P],
                                         rhs=w1_sb[:], start=True, stop=True)
                    x1_sb = tf.tile([P, 4 * h1], F32, tag="x1_sb")
                    nc.vector.tensor_copy(out=x1_sb[:], in_=ps[:])
                    # rows 512g+128i+p  <->  partition p, run i
                    nc.sync.dma_start(
                        out=x1_table[512 * g:512 * (g + 1), :].rearrange(
                            "(i p) d -> p i d", p=P),
                        in_=x1_sb[:].rearrange("p (i d) -> p i d", i=4))

            # ---- shared window-group machinery ----
            def group_aggregate(gidx, table, pools):
                """dma_gather + message scaling + one-hot build for pair gidx.
                Returns (Gs, S) tiles covering GRP windows x C blocks."""
                wp, h = pools
                idx_t = wp.tile([P, 16 * C], I16, tag="idx")
                nc.sync.dma_start(out=idx_t[:], in_=idx[gidx])
                ewd_t = wp.tile([P, 2 * W], BF16, tag="ewd")
                nc.sync.dma_start(out=ewd_t[:], in_=ewdst[gidx])

                g_t = wp.tile([P, W * h], F32, tag="gath")
                for j in range(nch):
                    nbt = GRP * B[j]
                    base = GRP * off[j]
                    for k in range(0, nbt, 8):  # <=1024 idxs per dma_gather
                        nb = min(8, nbt - k)
                        nc.gpsimd.dma_gather(
                            out_ap=g_t[:, (base + k) * h:(base + k + nb) * h]
                            .rearrange("p (b d) -> p b d", d=h),
                            in_ap=table[j * chunk_rows:(j + 1) * chunk_rows, :],
                            idxs_ap=idx_t[:, 8 * (base + k):8 * (base + k + nb)],
                            num_idxs=nb * P,
                            num_idxs_reg=nb * P,
                            elem_size=h,
                        )

                gs_t = wp.tile([P, W * h], BF16, tag="gs")
                nc.vector.tensor_tensor(
                    out=gs_t[:].rearrange("p (c d) -> p c d", d=h),
                    in0=g_t[:].rearrange("p (c d) -> p c d", d=h),
                    in1=ewd_t[:, 0:W, None].to_broadcast([P, W, h]),
                    op=mybir.AluOpType.mult)
                s_t = wp.tile([P, W * P], BF16, tag="sel")
                nc.vector.tensor_tensor(
                    out=s_t[:].rearrange("p (c j) -> p c j", j=P),
                    in0=iota128[:, None, :].to_broadcast([P, W, P]),
                    in1=ewd_t[:, W:2 * W, None].to_broadcast([P, W, P]),
                    op=mybir.AluOpType.is_equal)
                return gs_t, s_t

            def window_qlist(s):
                return [GRP * off[j] + s * B[j] + c
                        for j in range(nch) for c in range(B[j])]

            def accumulate(pp, gs_t, s_t, s, bias_rep, h):
                ps_z = pp.tile([P, h], F32, tag="ps_z")
                nc.tensor.matmul(ps_z[:], lhsT=ident[:], rhs=bias_rep[:],
                                 start=True, stop=False)
                qs = window_qlist(s)
                for n, q in enumerate(qs):
                    nc.tensor.matmul(ps_z[:],
                                     lhsT=s_t[:, q * P:(q + 1) * P],
                                     rhs=gs_t[:, q * h:(q + 1) * h],
                                     start=False, stop=(n == len(qs) - 1))
                return ps_z

            # ---- phase 2: layer-1 aggregation + per-window x2 transform ----
            with (
                tc.tile_pool(name="l1", bufs=3) as l1,
                tc.tile_pool(name="l1s", bufs=2) as l1s,
                tc.tile_pool(name="l1p", bufs=2, space="PSUM") as l1p,
                tc.tile_pool(name="l1tp", bufs=2, space="PSUM") as l1tp,
            ):
                for gi in range(npairs):
                    gs_t, s_t = group_aggregate(gi, x1_table, (l1, h1))
                    for s in range(GRP):
                        wi = gi * GRP + s
                        ps_z = accumulate(l1p, gs_t, s_t, s, b1rep, h1)
                        h_sb = l1s.tile([P, h1], BF16, tag="h_sb")
                        nc.scalar.activation(
                            out=h_sb[:], in_=ps_z[:],
                            func=mybir.ActivationFunctionType.Relu)
                        hT_ps = l1tp.tile([h1, P], BF16, tag="hT_ps")
                        nc.tensor.transpose(out=hT_ps[:], in_=h_sb[:],
                                            identity=ident[:])
                        hT_sb = l1s.tile([h1, P], BF16, tag="hT_sb")
                        nc.vector.tensor_copy(out=hT_sb[:], in_=hT_ps[:])
                        x2_ps = l1tp.tile([P, h2], F32, tag="x2_ps")
                        nc.tensor.matmul(x2_ps[:], lhsT=hT_sb[:], rhs=w2_sb[:],
                                         start=True, stop=True)
                        x2_sb = l1s.tile([P, h2], F32, tag="x2_sb")
                        nc.vector.tensor_copy(out=x2_sb[:], in_=x2_ps[:])
                        nc.sync.dma_start(
                            out=x2_slice[wi * P:(wi + 1) * P, :], in_=x2_sb[:])

            # ---- AllGather x2 slices -> full x2 table on every core ----
            nc.gpsimd.collective_compute(
                "AllGather", mybir.AluOpType.bypass,
                replica_groups=[list(range(NCORES))],
                ins=[x2_slice[:]], outs=[x2_table[:]])

            # ---- phase 3: layer-2 aggregation -> final output ----
            with (
                tc.tile_pool(name="l2", bufs=3) as l2,
                tc.tile_pool(name="l2s", bufs=2) as l2s,
                tc.tile_pool(name="l2p", bufs=4, space="PSUM") as l2p,
            ):
                for gi in range(npairs):
                    gs_t, s_t = group_aggregate(gi, x2_table, (l2, h2))
                    for s in range(GRP):
                        wi = gi * GRP + s
                        ps_z = accumulate(l2p, gs_t, s_t, s, b2rep, h2)
                        o_sb = l2s.tile([P, h2], F32, tag="o_sb")
                        nc.vector.tensor_copy(out=o_sb[:], in_=ps_z[:])
                        nc.sync.dma_start(out=out2[wi * P:(wi + 1) * P, :],
                                          in_=o_sb[:])

    nc.compile()
    return nc


def prep_inputs(edge_index, edge_weight, emb, W1, b1, W2, b2, npad, nch):
    """Host-side sharding: edges keyed by (dst window, src chunk), laid out
    into the per-pair-group block/partition slots the program expects."""
    wpc = npad // (P * NCORES)
    nwin = npad // P
    chunk_rows = npad // nch
    assert chunk_rows <= 32768
    n_in = emb.shape[1]
    src = np.asarray(edge_index[0], dtype=np.int64)
    dst = np.asarray(edge_index[1], dtype=np.int64)
    ew = np.asarray(edge_weight, dtype=np.float32)
    e = src.shape[0]

    w = dst >> 7
    j = src // chunk_rows
    key = w * nch + j
    cnt = np.bincount(key, minlength=nwin * nch).reshape(nwin, nch)
    B = [max(1, int(-(-cnt[:, jj].max() // P))) for jj in range(nch)]
    C = sum(B)
    off = np.concatenate([[0], np.cumsum(B)]).astype(np.int64)
    npairs_total = nwin // GRP
    Wt = GRP * C

    order = np.argsort(key, kind="stable")
    sk = key[order]
    starts = np.zeros(nwin * nch + 1, dtype=np.int64)
    starts[1:] = np.cumsum(cnt.reshape(-1))
    rank = np.arange(e, dtype=np.int64) - starts[sk]

    sw, sj = w[order], j[order]
    pg = sw // GRP           # pair-group id
    s = sw % GRP             # window within pair
    cc = rank // P           # block within (window, chunk)
    pp = rank % P
    q = GRP * off[sj] + s * B_arr(B)[sj] + cc

    ew_arr = np.zeros((npairs_total, P, 2 * Wt), np.float32)
    ew_arr[pg, pp, q] = ew[order]
    ew_arr[pg, pp, Wt + q] = (dst[order] % P).astype(np.float32)
    ewdst = ew_arr.astype(BF16_NP)

    # idx array: per (pair, chunk): flat order i = (s*B_j + c)*128 + p,
    # wrapped to [i % 16, 16*off_j + i // 16], replicated to 128 partitions.
    i_flat = (s * B_arr(B)[sj] + cc) * P + pp
    idx16 = np.zeros((npairs_total, 16, 8 * GRP * C), np.int16)
    idx16[pg, i_flat % 16, 8 * GRP * off[sj] + i_flat // 16] = \
        (src[order] - sj * chunk_rows).astype(np.int16)
    idx_full = np.tile(idx16, (1, 8, 1))

    embp = np.zeros((npad, n_in), np.float32)
    embp[:emb.shape[0]] = emb
    embT_bf = np.ascontiguousarray(embp.T).astype(BF16_NP)

    common = {
        "embT": embT_bf,
        "w1": np.asarray(W1, np.float32),
        "w2": np.asarray(W2, np.float32),
        "b1": np.asarray(b1, np.float32).reshape(1, -1),
        "b2": np.asarray(b2, np.float32).reshape(1, -1),
    }
    ppc = wpc // GRP  # pairs per core
    in_maps = []
    for k in range(NCORES):
        lo, hi = k * ppc, (k + 1) * ppc
        in_maps.append(dict(common,
                            idx=np.ascontiguousarray(idx_full[lo:hi]),
                            ewdst=np.ascontiguousarray(ewdst[lo:hi])))
    return in_maps, chunk_rows, B


def B_arr(B):
    return np.asarray(B, dtype=np.int64)


def run(edge_index, edge_weight, emb, W1, b1, W2, b2, npad, nch):
    n = emb.shape[0]
    wpc = npad // (P * NCORES)
    in_maps, chunk_rows, B = prep_inputs(
        edge_index, edge_weight, emb, W1, b1, W2, b2, npad, nch)
    nc = build_nc(npad, wpc, chunk_rows, B, emb.shape[1], W1.shape[1],
                  W2.shape[1])
    res = run_bass_kernel_spmd(nc, in_maps, core_ids=list(range(NCORES)))
    out = np.concatenate([res.results[k]["out2"] for k in range(NCORES)], axis=0)
    return out[:n]


def kernel(edge_index, edge_weight, emb, W1, b1, W2, b2):
    # 784 windows of 128 nodes; gather chunks of 25088 rows (int16-indexable)
    return run(edge_index, edge_weight, emb, W1, b1, W2, b2,
               npad=100352, nch=4)
